# revision 9
# baseline (speedup 1.0000x reference)
"""Multi-head attention Trainium2 kernel (nn_MultiHeadAttention_86423331930281).

Self-contained: data-parallel over batch (B=8 -> one batch element per
NeuronCore), runs on cores 0-7 via run_bass_kernel_spmd, returns the full
[8, 1024, 1024] output.

Per-core algorithm (S=1024, D=1024, H=16, E=64), all-bf16 matmul operands:
  - v/q/k: gpsimd cast-load fp32->bf16 two row-blocks per DMA, PE-transpose
    (bf16 identity, 1 cycle/row) into single [128, 8, S] tiles; one-bank
    row-block psum tiles give one evac per row (DVE/ACT alternating)
  - wo: gpsimd cast-load, store to DRAM bf16 scratch, xbar DMA-transpose
    back -> woT [he, out] (entirely off the critical path, SP-issued)
  - wv: gpsimd strided cast-load [ki, ko, h, e]; wq/wk: per-head-pair
    just-in-time gpsimd cast-loads, prefetched 3 pairs ahead
  - V1[t, h, e|1] = vT.T @ Wv with a trailing ones column per head
  - per head-pair: QT/KT [128=2*64, s] = Wq_pair-chunks.T @ qT (8-chunk
    accum in a dedicated psum pool, decoupled from the exp drain)
  - per head: scoresT [t, s] = KT_h-slices.T @ QT_h (K=64), exp on ACT
    (scale=1/32 folded) -> P [t, s] bf16
  - attended in [s, e] orientation with a ONE-HEAD SOFTWARE LAG: the
    previous head's chains att[s, 65] += P[t-chunk, s-chunk].T @ V1 are
    interleaved into the current head's score emission so they never wait
    on the serial exp stream; the 65th column accumulates the softmax
    denominator for free
  - normalize with per-partition reciprocal+multiply (denominator is a
    per-partition scalar in this orientation -- no broadcast round-trip),
    PE re-transpose [s,64]->[64,s] into attT [he, s]
  - FC split: FC1 = attT[m<7].T @ WoT runs during the last head's exp
    drain; partials (+bias, f32 DVE add) park in SBUF as bf16 and FC2's
    DVE evac adds them to the m=7 contribution -- no extra PE matmuls
"""

import numpy as np
from contextlib import ExitStack

import concourse.bass as bass
import concourse.mybir as mybir
import concourse.tile as tile
from concourse.bass_utils import run_bass_kernel_spmd
from concourse.masks import make_identity

P = 128
S = 1024          # sequence length
DK = 1024         # qkv input dim
H = 16            # heads
E = 64            # per-head dim
HE = H * E        # 1024
OUT = 1024        # output dim
NT = S // P       # 8 s/t tiles
NK = DK // P      # 8 contraction tiles
NM = H // 2       # 8 head pairs
F32 = mybir.dt.float32
BF16 = mybir.dt.bfloat16
AF = mybir.ActivationFunctionType
ALU = mybir.AluOpType
SCALE = 1.0 / 32.0  # 1/sqrt(DK)


def _legalize_matmul_waits(nc):
    """This walrus build allows only ONE sync-wait command per Matmult.
    Move all but the last wait of any multi-wait matmul onto freshly
    inserted PE nops immediately before it — same engine queue, so the
    blocking semantics are identical."""
    SKIP = ("NoOp", "Br", "Halt", "Sem", "Event")
    k = 0
    for f in nc.m.functions:
        for b in f.blocks:
            out = []
            for inst in b.instructions:
                si = getattr(inst, "sync_info", None)
                tname = type(inst).__name__
                if (not any(s in tname for s in SKIP) and si is not None
                        and si.on_wait and len(si.on_wait) > 1):
                    waits = list(si.on_wait)
                    for w in waits[:-1]:
                        nop = mybir.InstNoOp(
                            name=f"legalize-nop-{k}", ins=[], outs=[])
                        k += 1
                        nop.engine = inst.engine
                        nop.sync_info = mybir.SyncInfo(
                            on_wait=[w], on_update=[])
                        out.append(nop)
                    inst.sync_info = mybir.SyncInfo(
                        on_wait=[waits[-1]], on_update=list(si.on_update))
                out.append(inst)
            b.instructions[:] = out
    return k


def build(legalize=True):
    nc = bass.Bass()
    q_d = nc.dram_tensor("q", (S, DK), F32, kind="ExternalInput")
    k_d = nc.dram_tensor("k", (S, DK), F32, kind="ExternalInput")
    v_d = nc.dram_tensor("v", (S, DK), F32, kind="ExternalInput")
    wq_d = nc.dram_tensor("wq", (H, DK, E), F32, kind="ExternalInput")
    wk_d = nc.dram_tensor("wk", (H, DK, E), F32, kind="ExternalInput")
    wv_d = nc.dram_tensor("wv", (H, DK, E), F32, kind="ExternalInput")
    wo_d = nc.dram_tensor("wo", (OUT, HE), F32, kind="ExternalInput")
    bo_d = nc.dram_tensor("bo", (OUT,), F32, kind="ExternalInput")
    out_d = nc.dram_tensor("out", (S, OUT), F32, kind="ExternalOutput")
    wob_d = nc.dram_tensor("wob_scratch", (OUT, HE), BF16, kind="Internal")

    # [h, d, e] viewed as [di, ko, h, e] so partition = inner contraction dim
    wq_v = wq_d.rearrange("h (ko ki) e -> ki ko h e", ki=P)
    wk_v = wk_d.rearrange("h (ko ki) e -> ki ko h e", ki=P)
    wv_v = wv_d.rearrange("h (ko ki) e -> ki ko h e", ki=P)

    with tile.TileContext(nc) as tc, ExitStack() as ctx:
        const = ctx.enter_context(tc.tile_pool(name="const", bufs=1))
        src = ctx.enter_context(tc.tile_pool(name="src", bufs=4))
        xTq = ctx.enter_context(tc.tile_pool(name="xTq", bufs=1))
        woTp = ctx.enter_context(tc.tile_pool(name="woTp", bufs=NK))
        wqkp = ctx.enter_context(tc.tile_pool(name="wqkp", bufs=6))
        v1p = ctx.enter_context(tc.tile_pool(name="v1p", bufs=NT))
        # scores psum: 2 x 2 banks; proj/fc psum: 2 x 1 bank (decoupled so
        # projections never wait on the exp drain tail); the re-transpose
        # collect tiles share the proj slots (same tag, same bank size)
        scp = ctx.enter_context(tc.tile_pool(name="scp", bufs=2, space="PSUM"))
        pjp = ctx.enter_context(tc.tile_pool(name="pjp", bufs=2, space="PSUM"))
        ph1 = ExitStack()
        vTp = ph1.enter_context(tc.tile_pool(name="vTp", bufs=1))
        wvp = ph1.enter_context(tc.tile_pool(name="wvp", bufs=2))


        # ---- load + transpose phase --------------------------------------
        # one [128, NK, S] tile per transposed matrix; chunk j = [:, j, :]
        vTq = vTp.tile([P, NK, S], BF16, name="vT", tag="vT")
        qTq = xTq.tile([P, NK, S], BF16, name="qT", tag="qT")
        kTq = xTq.tile([P, NK, S], BF16, name="kT", tag="kT")
        woT = [woTp.tile([P, S], BF16, name=f"woT{j}", tag="woT")
               for j in range(NK)]

        tpp = ph1.enter_context(tc.tile_pool(name="tpp", bufs=2,
                                             space="PSUM"))

        NB = 2  # row-blocks per cast DMA (NB=4 halves the Pool dge count
        #         but crashes NRT execution; NB=2 is hardware-verified)

        def warmup(n):
            """dummy transposes keep the PE p-state ramp alive until the
            first real data lands (cost model runs full-rate only after
            3 us of continuous busy)"""
            dmy = tpp.tile([2, P], BF16, tag="tp", name="warm")
            for i in range(n):
                nc.tensor.transpose(dmy[0:2, :], ident_bf[:, 0:2],
                                    ident_bf[:])

        def emit_casts(mat_d, nm, n=None):
            """cast-load NB row-blocks per DMA"""
            stbs = []
            for rr in range(n if n is not None else NT // NB):
                stb = src.tile([P, NB, DK], BF16, tag="srcb",
                               name=f"{nm}cast{rr}")
                nc.gpsimd.dma_start(
                    stb[:],
                    mat_d[rr * NB * P:(rr + 1) * NB * P, :].rearrange(
                        "(c p) d -> p c d", c=NB))
                stbs.append(stb)
            return stbs

        def emit_transposes(stbs, tile, nm):
            """PE-transpose a full row-block into one 1-bank psum tile;
            single evac per row-block (DVE/ACT alternating — GPSIMD cannot
            read PSUM)"""
            for r in range(NT):
                stb = stbs[r // NB]
                c = r % NB
                # alternate with the (still idle) scores pool slots so the
                # evac WAR never paces the transposes
                pool, tg = (tpp, "tp") if r % 2 == 0 else (scp, "sc")
                pt_ = pool.tile([P, NK, P], BF16, tag=tg, name=f"{nm}ps{r}")
                for j in range(NK):
                    nc.tensor.transpose(
                        pt_[:, j, :], stb[:, c, j * P:(j + 1) * P],
                        ident_bf[:])
                if r % 2 == 0:
                    nc.vector.tensor_copy(
                        tile[:, :, r * P:(r + 1) * P], pt_[:])
                else:
                    nc.scalar.copy(
                        tile[:, :, r * P:(r + 1) * P], pt_[:])

        def pe_transpose(mat_d, tile, nm):
            emit_transposes(emit_casts(mat_d, nm), tile, nm)

        def xs(tile, j):
            """[128, S] view of transposed chunk j"""
            return tile[:, j, :]

        # first v cast goes out before the identity init so data and
        # identity land together for the first transpose
        v_stbs = emit_casts(v_d[0:NB * P, :], "v0", n=1)
        ident = const.tile([P, P], F32, name="ident")
        make_identity(nc, ident)
        ident_bf = const.tile([P, P], BF16, name="ident_bf")
        nc.vector.tensor_copy(ident_bf[:], ident[:])
        v_stbs += emit_casts(v_d[NB * P:, :], "v1", n=3)
        warmup(56)
        emit_transposes(v_stbs, vTq, "v")

        # wv: strided cast-load [ki, ko, h, e] per contraction chunk
        wv_sb = []
        for half in range(2):
            t = wvp.tile([P, NK // 2, H, E], BF16, tag="wwv",
                         name=f"wvsb{half}")
            for jj in range(NK // 2):
                nc.gpsimd.dma_start(
                    t[:, jj], wv_v[:, half * (NK // 2) + jj])
            wv_sb.append(t)

        def prefetch_w(m):
            """per-pair just-in-time Wq/Wk chunk loads [ki, ko, 2, e]"""
            wqm = wqkp.tile([P, NK, 2, E], BF16, tag="wqk", name=f"wqm{m}")
            wkm = wqkp.tile([P, NK, 2, E], BF16, tag="wqk", name=f"wkm{m}")
            for hh in range(2):
                nc.gpsimd.dma_start(wqm[:, :, hh, :], wq_v[:, :, 2 * m + hh, :])
                nc.gpsimd.dma_start(wkm[:, :, hh, :], wk_v[:, :, 2 * m + hh, :])
            return wqm, wkm

        w_pref = {0: prefetch_w(0)}

        # ---- V projection: V1 [t, h, e|ones] ------------------------------
        v1_tiles = []
        for i in range(NT):
            v1 = v1p.tile([P, H, E + 1], BF16, tag="v1", name=f"v1_{i}")
            nc.gpsimd.memset(v1[:, :, E], 1.0)
            for nh in range(2):
                pst = pjp.tile([P, 512], F32, tag="pj", name=f"vproj{i}_{nh}")
                for j in range(NK):
                    wvf = wv_sb[j // (NK // 2)][:, j % (NK // 2)].rearrange(
                        "p h e -> p (h e)")
                    nc.tensor.matmul(
                        pst[:],
                        xs(vTq, j)[:, i * P:(i + 1) * P],
                        wvf[:, nh * 512:(nh + 1) * 512],
                        start=(j == 0), stop=(j == NK - 1))
                nc.vector.tensor_copy(
                    v1[:, nh * (H // 2):(nh + 1) * (H // 2), 0:E],
                    pst[:].rearrange("p (h e) -> p h e", e=E))
            v1_tiles.append(v1)

        w_pref[1] = prefetch_w(1)
        pe_transpose(q_d, qTq, "q")
        w_pref[2] = prefetch_w(2)
        pe_transpose(k_d, kTq, "k")

        # FC-only constant, emitted after the critical-path loads (SP HWDGE)
        bo_bc = const.tile([P, OUT], F32, name="bo_bc")
        nc.sync.dma_start(bo_bc[:], bo_d[None, :].to_broadcast((P, OUT)))

        ph1.close()

        # ---- attention (one-head software pipeline lag) -------------------
        qtp = ctx.enter_context(tc.tile_pool(name="qtp", bufs=4))
        ptp = ctx.enter_context(tc.tile_pool(name="ptp", bufs=2 * NT))
        normp = ctx.enter_context(tc.tile_pool(name="normp", bufs=16))
        denp = ctx.enter_context(tc.tile_pool(name="denp", bufs=8))
        attp = ctx.enter_context(tc.tile_pool(name="attp", bufs=NM))
        att_ps = ctx.enter_context(
            tc.tile_pool(name="att_ps", bufs=2, space="PSUM"))

        attT_tiles = [attp.tile([P, S], BF16, tag="attT", name=f"attT{m}")
                      for m in range(NM)]

        # wo: cast-load bf16 (gpsimd, queued after the critical-path loads),
        # store to scratch (SP), xbar DMA-transpose back (SP) — SP is
        # otherwise idle until the out writes; needed only by the FC
        for rr in range(NT // 2):
            stb = src.tile([P, 2, DK], BF16, tag="srcb", name=f"wocast{rr}")
            nc.gpsimd.dma_start(
                stb[:],
                wo_d[rr * 2 * P:(rr + 1) * 2 * P, :].rearrange(
                    "(c p) d -> p c d", c=2))
            for c in range(2):
                r = rr * 2 + c
                nc.sync.dma_start(wob_d[r * P:(r + 1) * P, :], stb[:, c, :])
        for j in range(NK):
            nc.sync.dma_start_transpose(
                woT[j][:], wob_d[:, j * P:(j + 1) * P])

        def emit_att(h, ptiles, si):
            """attended [s-chunk si, e|denom] for head h + normalize"""
            if si % 2 == 0:
                _att_slot[0] = att_ps.tile([P, 2, E + 1], F32, tag="attps",
                                           name=f"att{h}_{si}")
            aps = _att_slot[0][:, si % 2, :]
            for j in range(NT):
                nc.tensor.matmul(
                    aps[0:P, 0:E + 1],
                    ptiles[j][:, si * P:(si + 1) * P],
                    v1_tiles[j][:, h, :],
                    start=(j == 0), stop=(j == NT - 1))
            den = denp.tile([P, 1], F32, tag="den", name=f"den{h}_{si}")
            nc.vector.reciprocal(den[:], aps[0:P, E:E + 1])
            nrm = normp.tile([P, E], BF16, tag="nrm", name=f"nrm{h}_{si}")
            nc.vector.tensor_scalar(nrm[:], aps[0:P, 0:E], den[:], None,
                                    ALU.mult)
            return nrm

        _att_slot = [None]
        pend = None  # (m, hs, nrm list) awaiting re-transpose + evac

        def flush_pend():
            nonlocal pend
            if pend is None:
                return
            pm, phs, ph_, nrms = pend
            tph = pjp.tile([E, S], BF16, tag="pj", name=f"tph{ph_}")
            for si in range(NT):
                nc.tensor.transpose(tph[:, si * P:(si + 1) * P], nrms[si][:],
                                    ident_bf[:])
            nc.vector.tensor_copy(attT_tiles[pm][phs, :], tph[:])
            pend = None

        prev_att = None  # (h, ptiles) whose attended chains interleave next

        for m in range(NM):
            if m + 3 < NM:
                w_pref[m + 3] = prefetch_w(m + 3)
            wqm, wkm = w_pref.pop(m)

            # QT_m / KT_m: [he_pair=128, s=1024], evacuated as bf16
            qkm = []
            for wm, xtiles, lbl in ((wqm, qTq, "qtm"), (wkm, kTq, "ktm")):
                t = qtp.tile([P, S], BF16, tag="qt", name=f"{lbl}{m}")
                for sh in range(2):
                    pst = pjp.tile([P, 512], F32, tag="pj",
                                   name=f"{lbl}ps{m}_{sh}")
                    for j in range(NK):
                        nc.tensor.matmul(
                            pst[:],
                            wm[:, j],
                            xs(xtiles, j)[:, sh * 512:(sh + 1) * 512],
                            start=(j == 0), stop=(j == NK - 1))
                    nc.vector.tensor_copy(t[:, sh * 512:(sh + 1) * 512],
                                          pst[:])
                qkm.append(t)
            qtm, ktm = qkm

            for hh in range(2):
                h = 2 * m + hh
                hs = slice(hh * E, (hh + 1) * E)
                # scoresT + exp -> P_j [t, s] bf16, with the previous head's
                # attended chains interleaved (their exps are already done)
                ptiles = []
                for j in range(NT):
                    pt = ptp.tile([P, S], BF16, tag="pt", name=f"p{h}_{j}")
                    sc = scp.tile([P, S], F32, tag="sc", name=f"sc{h}_{j}")
                    for sh in range(2):
                        nc.tensor.matmul(
                            sc[:, sh * 512:(sh + 1) * 512],
                            ktm[hs, j * P:(j + 1) * P],
                            qtm[hs, sh * 512:(sh + 1) * 512],
                            start=True, stop=True)
                    nc.scalar.activation(pt[:], sc[:], AF.Exp, scale=SCALE)
                    ptiles.append(pt)
                    if prev_att is not None:
                        nrm = emit_att(prev_att[0], prev_att[1], j)
                        prev_att[2].append(nrm)
                if prev_att is not None:
                    ph_, ppt, nrms = prev_att
                    flush_pend()
                    pend = (ph_ // 2, slice((ph_ % 2) * E, (ph_ % 2 + 1) * E),
                            ph_, nrms)
                prev_att = (h, ptiles, [])

        # ---- drain + FC, software-pipelined ------------------------------
        # FC1 (heads of pairs 0..6) runs while the last head's exps drain;
        # its partials (+bias) park in SBUF. The tail is then only the last
        # head's attended, its transposes, and a single-matmul FC2 pass.
        outp = ctx.enter_context(tc.tile_pool(name="outp", bufs=8))
        fc1p = ctx.enter_context(tc.tile_pool(name="fc1p", bufs=2 * NT))
        ph_, ppt, nrms = prev_att
        flush_pend()
        chunks = [(st, oh) for st in range(NT) for oh in range(2)]
        fc1_sb = [None] * len(chunks)

        def emit_fc1(ci):
            st, oh = chunks[ci]
            # 4-slot rotation with FC2 across both psum pools (scores pool
            # is idle during the drain) so evac latency never gates PE
            pool, tag = (pjp, "pj") if ci % 2 == 0 else (scp, "sc")
            pso = pool.tile([P, 512], F32, tag=tag, name=f"fc1_{st}_{oh}")
            for m in range(NM - 1):
                nc.tensor.matmul(
                    pso[:],
                    attT_tiles[m][:, st * P:(st + 1) * P],
                    woT[m][:, oh * 512:(oh + 1) * 512],
                    start=(m == 0), stop=(m == NM - 2))
            t = fc1p.tile([P, 512], BF16, tag="fc1", name=f"fc1sb{st}_{oh}")
            # DVE evac folds the bias in; the last head's normalize chain is
            # already drained by the early att emission, so no hostage stall
            nc.vector.tensor_tensor(
                t[:], pso[:], bo_bc[:, oh * 512:(oh + 1) * 512], ALU.add)
            fc1_sb[ci] = t

        def emit_fc2(ci):
            st, oh = chunks[ci]
            pool, tag = (scp, "sc") if ci % 2 == 0 else (pjp, "pj")
            pso = pool.tile([P, 512], F32, tag=tag, name=f"fc2_{st}_{oh}")
            nc.tensor.matmul(
                pso[:],
                attT_tiles[NM - 1][:, st * P:(st + 1) * P],
                woT[NM - 1][:, oh * 512:(oh + 1) * 512],
                start=True, stop=True)
            ot = outp.tile([P, 512], F32, tag="out", name=f"out{st}_{oh}")
            nc.vector.tensor_tensor(ot[:], pso[:], fc1_sb[ci][:], ALU.add)
            nc.sync.dma_start(
                out_d[st * P:(st + 1) * P, oh * 512:(oh + 1) * 512], ot[:])

        # the last head's attended chains go out immediately (their exps
        # finish during the first FC1 chunk), so attT completes early and
        # the serialized out-DMA stream can start near the drain's front
        emit_fc1(0)
        emit_fc1(1)
        for si in range(NT):
            nrms.append(emit_att(ph_, ppt, si))
            if si % 2 == 1 and 2 + si // 2 < len(chunks):
                emit_fc1(2 + si // 2)
        pend = (ph_ // 2, slice((ph_ % 2) * E, (ph_ % 2 + 1) * E), ph_, nrms)
        flush_pend()
        for ci in range(6, len(chunks)):
            emit_fc1(ci)
            emit_fc2(ci - 6)
        for ci in range(len(chunks) - 6, len(chunks)):
            emit_fc2(ci)
    if legalize:
        _legalize_matmul_waits(nc)
    return nc


_NC_CACHE = {}


def _get_nc():
    if "nc" not in _NC_CACHE:
        _NC_CACHE["nc"] = build()
    return _NC_CACHE["nc"]


def kernel(query, key, value, Wq, Wk, Wv, Wo, bo, **run_kwargs):
    query = np.asarray(query, dtype=np.float32)
    key = np.asarray(key, dtype=np.float32)
    value = np.asarray(value, dtype=np.float32)
    Wq = np.ascontiguousarray(np.asarray(Wq, dtype=np.float32))
    Wk = np.ascontiguousarray(np.asarray(Wk, dtype=np.float32))
    Wv = np.ascontiguousarray(np.asarray(Wv, dtype=np.float32))
    Wo = np.ascontiguousarray(np.asarray(Wo, dtype=np.float32))
    bo = np.ascontiguousarray(np.asarray(bo, dtype=np.float32))
    B = query.shape[0]
    assert B == 8, f"expected batch 8, got {B}"

    nc = _get_nc()
    in_maps = []
    for b in range(B):
        in_maps.append({
            "q": np.ascontiguousarray(query[b]),
            "k": np.ascontiguousarray(key[b]),
            "v": np.ascontiguousarray(value[b]),
            "wq": Wq, "wk": Wk, "wv": Wv, "wo": Wo, "bo": bo,
        })
    res = run_bass_kernel_spmd(nc, in_maps, core_ids=list(range(B)),
                               **run_kwargs)
    out = np.stack([r["out"] for r in res.results], axis=0)
    if run_kwargs.get("trace"):
        _NC_CACHE["last_result"] = res
    return out
